# revision 1
# baseline (speedup 1.0000x reference)
"""Trainium2 Bass kernel for a 2-layer LSTM + dense head (batch-sharded over 8 cores).

Reference computation (PyTorch gate order i,f,g,o):
  h1 = LSTM(x;   w_ih1, w_hh1, b_ih1+b_hh1)   # D=128 -> H1=128
  h2 = LSTM(h1;  w_ih2, w_hh2, b_ih2+b_hh2)   # H1=128 -> H2=64
  out = relu(h2[:, -1] @ w_dense.T + b_dense) # [B, 64]

Device-side design (per core, B_c = 256 batch columns):
  - All state "transposed": hidden-dim on SBUF partitions, batch on free dim.
  - fp16 everywhere in SBUF (x, weights, states, gate outputs); fp32 in PSUM.
    fp16 matmuls run at 1 cycle/row; fp16 doubles DVE tensor_tensor rate.
  - Gates i,f,o use Sigmoid directly; the g gate's tanh is computed as
    tanh(z) = 2*sigmoid(2z) - 1 (g weights doubled on host), so ALL four
    gates go through a single Sigmoid ACT op per layer, and every
    elementwise op is a 2-input tensor_tensor (fp16 2x on DVE):
      sg    = sigmoid(psum[i | f | o | 2g])   # one ACT op [128,1024]
      gt    = 2*sg_g - 1                      # tensor_scalar = tanh(g)
      v     = sg_f * c ;  u = sg_i * gt ;  c' = u + v
      thc   = tanh(c') ;  h' = sg_o * thc
    (Sigmoid, Tanh, Relu all live in one HW activation table -> no reloads.)
  - Layer 2 runs one timestep BEHIND layer 1 (software pipeline) so the two
    recurrence chains overlap.  Its v2 multiply runs on GPSIMD to keep the
    DVE queue tight.  (All elementwise operands share base partition 0 --
    the neuronxcc verifier rejects split-base accesses.)
  - Layer-1 input+bias matmuls for step t+1 are issued during step t into the
    other PSUM buffer (bufs=2), so only the 4 hidden matmuls precede the gate
    activation on the critical path.
"""

import os
import numpy as np

import concourse.bass as bass
import concourse.mybir as mybir
from concourse import bacc
from concourse.tile import TileContext
from concourse.bass_utils import run_bass_kernel_spmd

N_CORES = 8
B, T, D = 2048, 128, 128
H1, H2, OUT = 128, 64, 64
BC = B // N_CORES  # 256 batch per core
X_CHUNKS = [(0, 4), (4, 16), (16, 48), (48, 128)]  # staged x DMA (ramp-friendly)

FP = mybir.dt.float32
F16 = mybir.dt.float16
AF = mybir.ActivationFunctionType
ALU = mybir.AluOpType

# packed-weight column offsets: one DMA loads every constant (9 separate
# DMAs cost ~1.2us of DGE fixed overhead each on the startup ramp)
OFF_W1, OFF_B1 = 0, 512
OFF_ONES, OFF_WZ = 1024, 1280
WCRIT = 1408  # end of the step-0-critical slice (first DMA)
OFF_WH1, OFF_W2 = 1408, 1920
OFF_WH2, OFF_B2 = 2176, 2432
OFF_WD, OFF_BD = 2688, 2752
WCOLS = 2816

_PROGRAM_CACHE = {}


def build_program():
    if "nc" in _PROGRAM_CACHE:
        return _PROGRAM_CACHE["nc"]

    nc = bacc.Bacc(
        "TRN2", target_bir_lowering=False, debug=False,
        enable_asserts=False, num_devices=N_CORES,
    )

    # ---- DRAM parameters (per-core shapes; in_maps supply per-core data)
    xT_d = nc.declare_dram_parameter("xT", [D, T, BC], F16, isOutput=False)
    wpack_d = nc.declare_dram_parameter("wpack", [D, WCOLS], F16, isOutput=False)
    out_d = nc.declare_dram_parameter("outT", [OUT, BC], FP, isOutput=True)

    with TileContext(nc, num_cores=N_CORES) as tc:
        with (
            tc.tile_pool(name="const", bufs=1) as cpool,
            tc.tile_pool(name="actsbig", bufs=6) as bpool,
            tc.tile_pool(name="acts", bufs=20) as apool,
            tc.tile_pool(name="state", bufs=6) as spool,
            tc.tile_pool(name="ps1a", bufs=2, space="PSUM") as ps1apool,
            tc.tile_pool(name="ps1b", bufs=2, space="PSUM") as ps1bpool,
            tc.tile_pool(name="ps2a", bufs=2, space="PSUM") as ps2apool,
            tc.tile_pool(name="ps2b", bufs=2, space="PSUM") as ps2bpool,
        ):
            # ---- load all constants / weights with ONE DMA
            wpack = cpool.tile([D, WCOLS], F16, tag="wpack")
            nc.sync.dma_start(out=wpack[:, 0:OFF_B1], in_=wpack_d[:, 0:OFF_B1])
            w1c = lambda j: wpack[:, OFF_W1 + j * H1:OFF_W1 + (j + 1) * H1]
            wh1c = lambda j: wpack[:, OFF_WH1 + j * H1:OFF_WH1 + (j + 1) * H1]
            b1c = lambda j: wpack[0:1, OFF_B1 + j * H1:OFF_B1 + (j + 1) * H1]
            w2c = lambda k: wpack[:, OFF_W2 + k * 2 * H2:OFF_W2 + (k + 1) * 2 * H2]
            wh2c = lambda k: wpack[0:H2, OFF_WH2 + k * 2 * H2:OFF_WH2 + (k + 1) * 2 * H2]
            b2c = lambda k: wpack[0:1, OFF_B2 + k * 2 * H2:OFF_B2 + (k + 1) * 2 * H2]
            wdA = wpack[0:H2, OFF_WD:OFF_WD + OUT]
            bdA = wpack[0:1, OFF_BD:OFF_BD + OUT]
            onesA = wpack[0:1, OFF_ONES:OFF_ONES + BC]
            wzeroA = wpack[0:1, OFF_WZ:OFF_WZ + 2 * H2]

            xs = cpool.tile([D, T, BC], F16, tag="xs")
            a0, b0 = X_CHUNKS[0]
            nc.sync.dma_start(out=xs[:, a0:b0, :], in_=xT_d[:, a0:b0, :])
            nc.sync.dma_start(out=wpack[:, OFF_B1:WCRIT], in_=wpack_d[:, OFF_B1:WCRIT])
            nc.sync.dma_start(out=wpack[:, WCRIT:], in_=wpack_d[:, WCRIT:])
            for a, b_ in X_CHUNKS[1:]:
                nc.sync.dma_start(out=xs[:, a:b_, :], in_=xT_d[:, a:b_, :])

            czero = cpool.tile([H1, BC], F16, tag="czero")
            nc.vector.memset(czero[:], 0.0)

            h1p = c1p = h2p = c2p = None  # previous-step states

            def l1_prefetch(t):
                """input+bias matmuls for L1 step t into two fresh PSUM tiles:
                pa = (i, 2g), pb = (f, o). Separate tiles keep the two gate
                ACT halves fully decoupled in the scheduler."""
                pa = ps1apool.tile([H1, 2, BC], FP, tag="p1a")
                pb = ps1bpool.tile([H1, 2, BC], FP, tag="p1b")
                xt = xs[:, t, :]
                last = t == 0  # no hidden matmuls at t=0 -> stops live here
                for j, p in ((0, pa), (1, pa), (2, pb), (3, pb)):
                    nc.tensor.matmul(p[:, j % 2, :], w1c(j),
                                     xt, start=(j in (0, 2)), stop=False)
                for j, p in ((0, pa), (1, pa), (2, pb), (3, pb)):
                    nc.tensor.matmul(p[:, j % 2, :], b1c(j),
                                     onesA, start=False,
                                     stop=(last and j in (1, 3)))
                return pa, pb

            def l2_matmuls(s, h1s, dep_ap=None):
                """all matmuls for L2 step s (input from h1s, hidden from h2p).
                Gates packed pairwise on 128 partitions across two separate
                PSUM tiles: pa=[f|2g], pb=[i|o].  Each tile gets its own
                closing zero-weight matmul folding in dep_ap (if given) as a
                pure scheduling dependency, so each L2 gate-ACT half waits
                only its own column and never precedes the first L1 half."""
                pa = ps2apool.tile([2 * H2, BC], FP, tag="p2a")
                pb = ps2bpool.tile([2 * H2, BC], FP, tag="p2b")
                gate = dep_ap is not None
                for k, p in ((0, pa), (1, pb)):
                    nc.tensor.matmul(p[:], w2c(k), h1s[:],
                                     start=True, stop=False)
                for k, p in ((0, pa), (1, pb)):
                    nc.tensor.matmul(p[:], b2c(k), onesA,
                                     start=False, stop=(not gate and s == 0))
                if s > 0:
                    for k, p in ((0, pa), (1, pb)):
                        nc.tensor.matmul(p[:], wh2c(k), h2p[:],
                                         start=False, stop=(not gate))
                if gate:
                    nc.tensor.matmul(pa[:], wzeroA, dep_ap, start=False, stop=True)
                    nc.tensor.matmul(pb[:], wzeroA, dep_ap, start=False, stop=True)
                return pa, pb

            p1a, p1b = l1_prefetch(0)
            for t in range(T + 1):
                s = t - 1  # L2 step handled this iteration
                if t < T:
                    # -- PE: L1 hidden matmuls for step t (chain-critical).
                    # Gate order (i, 2g, f, o): the (i,g) tile closes after two
                    # matmuls so its gate-ACT half starts early; both of u's
                    # inputs are in that half, so the gt/u subchain overlaps
                    # the second half.
                    if t > 0:
                        for j, p in ((0, p1a), (1, p1a), (2, p1b), (3, p1b)):
                            nc.tensor.matmul(p[:, j % 2, :], wh1c(j),
                                             h1p[:], start=False, stop=(j in (1, 3)))

                    # -- ACT: L1 gates in two halves (i,g) then (f,o)
                    sga = bpool.tile([H1, 2, BC], F16, tag="sga")
                    sgb = bpool.tile([H1, 2, BC], F16, tag="sgb")
                    nc.scalar.activation(sga[:], p1a[:], AF.Sigmoid)
                    nc.scalar.activation(sgb[:], p1b[:], AF.Sigmoid)

                # -- PE: all matmuls for L2 step s (off-chain)
                if s >= 0:
                    p2a, p2b = l2_matmuls(s, h1p,
                                          dep_ap=sga[0:1, 0, :] if t < T else None)

                if t < T:
                    # -- DVE: L1 cell update (gt = tanh(g) = 2*sg_g - 1)
                    gt = apool.tile([H1, BC], F16, tag="gt")
                    v = apool.tile([H1, BC], F16, tag="v")
                    u = apool.tile([H1, BC], F16, tag="u")
                    c1n = spool.tile([H1, BC], F16, tag="c1")
                    nc.vector.tensor_scalar(gt[:], sga[:, 1, :], 2.0, 1.0,
                                            op0=ALU.mult, op1=ALU.subtract)
                    nc.vector.tensor_tensor(u[:], sga[:, 0, :], gt[:], op=ALU.mult)
                    nc.vector.tensor_tensor(v[:], sgb[:, 0, :],
                                            czero[:] if t == 0 else c1p[:], op=ALU.mult)
                    nc.vector.tensor_tensor(c1n[:], u[:], v[:], op=ALU.add)

                if s >= 0:
                    # -- ACT: L2 gates in two packed halves: a=[f|2g], b=[i|o].
                    # The (f,2g) half feeds the long poles (Pool v2, gt2) first.
                    sg2a = apool.tile([2 * H2, BC], F16, tag="sg2a")
                    sg2b = apool.tile([2 * H2, BC], F16, tag="sg2b")
                    nc.scalar.activation(sg2a[:], p2a[:], AF.Sigmoid)
                    nc.scalar.activation(sg2b[:], p2b[:], AF.Sigmoid)
                    # -- POOL: v2 (off the DVE queue; operands at base 0)
                    v2 = apool.tile([H2, BC], F16, tag="v2")
                    nc.vector.tensor_tensor(v2[:], sg2a[0:H2, :],
                                            czero[0:H2, :] if s == 0 else c2p[:],
                                            op=ALU.mult)
                    # -- DVE: gt2/so2 shift partitions 64-127 down to base 0
                    # (single-input ops may shift; two-input ops may not)
                    gt2 = apool.tile([H2, BC], F16, tag="gt2")
                    so2 = apool.tile([H2, BC], F16, tag="so2")
                    u2 = apool.tile([H2, BC], F16, tag="u2")
                    nc.vector.tensor_scalar(gt2[:], sg2a[H2:2 * H2, :], 2.0, 1.0,
                                            op0=ALU.mult, op1=ALU.subtract)
                    nc.vector.tensor_scalar(so2[:], sg2b[H2:2 * H2, :], 1.0, 0.0,
                                            op0=ALU.mult, op1=ALU.add)
                    nc.vector.tensor_tensor(u2[:], sg2b[0:H2, :], gt2[:], op=ALU.mult)

                if t < T:
                    # -- ACT: thc1 (chain), then DVE: h1n (chain)
                    thc1 = apool.tile([H1, BC], F16, tag="thc1")
                    nc.scalar.activation(thc1[:], c1n[:], AF.Tanh)
                    h1n = spool.tile([H1, BC], F16, tag="h1")
                    nc.vector.tensor_tensor(h1n[:], sgb[:, 1, :], thc1[:], op=ALU.mult)

                if s >= 0:
                    # -- DVE: c2n; ACT: thc2; DVE: h2n
                    c2n = spool.tile([H2, BC], F16, tag="c2")
                    nc.vector.tensor_tensor(c2n[:], u2[:], v2[:], op=ALU.add)
                    thc2 = apool.tile([H2, BC], F16, tag="thc2")
                    nc.scalar.activation(thc2[:], c2n[:], AF.Tanh)
                    h2n = spool.tile([H2, BC], F16, tag="h2")
                    nc.vector.tensor_tensor(h2n[:], so2[:], thc2[:], op=ALU.mult)
                    h2p, c2p = h2n, c2n

                # -- PE: prefetch L1 input+bias for step t+1
                if t < T - 1:
                    p1_next = l1_prefetch(t + 1)

                if t < T:
                    c1p, h1p = c1n, h1n
                if t < T - 1:
                    p1a, p1b = p1_next

            # ---- dense head on h2[T-1]
            pd = ps2apool.tile([OUT, BC], FP, tag="p2a")
            nc.tensor.matmul(pd[:], bdA, onesA, start=True, stop=False)
            nc.tensor.matmul(pd[:], wdA, h2p[:], start=False, stop=True)
            outs = cpool.tile([OUT, BC], FP, tag="outs")
            nc.scalar.activation(outs[:], pd[:], AF.Relu)
            nc.sync.dma_start(out=out_d[:], in_=outs[:])

    nc.finalize()
    _PROGRAM_CACHE["nc"] = nc
    return nc


def _prep_inputs(x, w_ih1, w_hh1, b_ih1, b_hh1, w_ih2, w_hh2, b_ih2, b_hh2,
                 w_dense, b_dense):
    """Host-side layout prep (fp16). Device gate order: [i, f, o, 2g] for L1;
    packed [i | 2g], [f | o] columns for L2. g weights doubled because
    tanh(z) = 2*sigmoid(2z) - 1 on device."""
    f16 = np.float16

    def gates(w_t, H):  # w_t: [in, 4H] torch order (i,f,g,o)
        i, f, g, o = (np.float64(w_t[:, k * H:(k + 1) * H]) for k in range(4))
        return i, f, 2.0 * g, o

    def cat(parts):
        return np.concatenate(parts, axis=-1).astype(f16)

    i1, f1, g1, o1 = gates(w_ih1.T, H1)
    w1 = cat([i1, g1, f1, o1])
    i1, f1, g1, o1 = gates(w_hh1.T, H1)
    wh1 = cat([i1, g1, f1, o1])
    i1, f1, g1, o1 = gates((b_ih1 + b_hh1)[None, :], H1)
    b1 = cat([i1, g1, f1, o1])

    i2, f2, g2, o2 = gates(w_ih2.T, H2)
    w2 = cat([f2, g2, i2, o2])
    i2, f2, g2, o2 = gates(w_hh2.T, H2)
    wh2 = cat([f2, g2, i2, o2])
    i2, f2, g2, o2 = gates((b_ih2 + b_hh2)[None, :], H2)
    b2 = cat([f2, g2, i2, o2])

    wd = np.float64(w_dense.T).astype(f16)
    bd = b_dense.astype(f16)[None, :]

    wpack = np.zeros((D, WCOLS), f16)
    wpack[:, OFF_W1:OFF_W1 + 4 * H1] = w1
    wpack[:, OFF_WH1:OFF_WH1 + 4 * H1] = wh1
    wpack[:, OFF_W2:OFF_W2 + 4 * H2] = w2
    wpack[0:H2, OFF_WH2:OFF_WH2 + 4 * H2] = wh2
    wpack[0:1, OFF_B1:OFF_B1 + 4 * H1] = b1
    wpack[0:1, OFF_B2:OFF_B2 + 4 * H2] = b2
    wpack[0:H2, OFF_WD:OFF_WD + OUT] = wd
    wpack[0:1, OFF_BD:OFF_BD + OUT] = bd
    wpack[0:1, OFF_ONES:OFF_ONES + BC] = 1.0

    xT = np.asarray(x, dtype=f16).transpose(2, 1, 0)  # [D,T,B]
    shared = dict(wpack=wpack)
    in_maps = []
    for c in range(N_CORES):
        m = dict(shared)
        m["xT"] = np.ascontiguousarray(xT[:, :, c * BC:(c + 1) * BC])
        in_maps.append(m)
    return in_maps


def _run(inputs, trace=False, **kw):
    nc = build_program()
    in_maps = _prep_inputs(**inputs)
    res = run_bass_kernel_spmd(nc, in_maps, list(range(N_CORES)), trace=trace, **kw)
    out = np.concatenate([np.asarray(res.results[c]["outT"]).T for c in range(N_CORES)], axis=0)
    return out.astype(np.float32), res


def kernel(**inputs):
    out, _ = _run(inputs, trace=False)
    return out


if __name__ == "__main__":
    import reference
    inputs = {k: np.asarray(v) for k, v in reference.setup_inputs().items()}
    expected = np.asarray(reference.reference(**inputs))
    out, res = _run(inputs, trace=os.environ.get("KTRACE", "0") == "1")
    err = np.abs(out - expected)
    rel = err.max() / (np.abs(expected).max() + 1e-12)
    print("max abs err:", err.max(), "rel:", rel)
    print("exec_time_ns:", res.exec_time_ns)



# revision 21
# speedup vs baseline: 1.0102x; 1.0102x over previous
"""Trainium2 Bass kernel for a 2-layer LSTM + dense head (batch-sharded over 8 cores).

Reference computation (PyTorch gate order i,f,g,o):
  h1 = LSTM(x;   w_ih1, w_hh1, b_ih1+b_hh1)   # D=128 -> H1=128
  h2 = LSTM(h1;  w_ih2, w_hh2, b_ih2+b_hh2)   # H1=128 -> H2=64
  out = relu(h2[:, -1] @ w_dense.T + b_dense) # [B, 64]

Device-side design (per core, BC=256 batch, split into 2 chunks of 128):
  - All state "transposed": hidden-dim on SBUF partitions, batch on free dim.
    fp16 in SBUF; fp32 PSUM.
  - Two independent batch chunks (A=cols 0:128, B=128:256) are interleaved so
    one chunk's compute hides the other's recurrence latency.
  - All sigmoids/tanhs run through the Sigmoid table only:
      tanh(z) = 2*sigmoid(2z) - 1 (g weights/biases doubled on host)
      cell state kept as c' = 2c, so tanh(c) = 2*sigmoid(c') - 1
      h kept halved:  h/2 = (sigmoid(c') - 0.5) * sigmoid_o  (one fused STT)
    All weights that consume h (wh1, w2, wh2, wd) are doubled on host.
  - L1 per chunk per step: one sigmoid ACT [128, 4*128] (i,2g,f,o), DVE cell
      t1 = (sg_g - 0.5)*sg_i ; v = sg_f*c'p ; c' = 4*t1 + v,
    one small sigmoid ACT [128,128] on c', one STT for h/2.
  - L2 is a LAZY pipeline two steps behind L1, fully decoupled from the L1
    recurrence: both chunks' gates go to ONE shared PSUM tile ([128, 2,4,64],
    "Q" layout: partition = (batch-half-of-chunk)*64 + h2_channel) so one
    sigmoid ACT covers both chunks; its cell ops run on GPSIMD; its c'-sigmoid
    and h2 update happen one iteration later (the slack exists by construction).
  - Biases are added via tiny PE matmuls against a ones-vector (ACT
    per-partition bias can't vary per gate within one merged ACT).
"""

import os
import numpy as np

import concourse.bass as bass
import concourse.mybir as mybir
from concourse import bacc
from concourse.tile import TileContext
from concourse.bass_utils import run_bass_kernel_spmd

N_CORES = 8
B, T, D = 2048, 128, 128
H1, H2, OUT = 128, 64, 64
BC = B // N_CORES   # 256 batch per core
CH = BC // 2        # 128 batch per chunk
QH = CH // 2        # 64 batch per Q-half
X_CHUNKS = [(0, 4), (4, 16), (16, 48), (48, 128)]  # staged x DMA

FP = mybir.dt.float32
F16 = mybir.dt.float16
AF = mybir.ActivationFunctionType
ALU = mybir.AluOpType

L1G = lambda j: slice(j * CH, (j + 1) * CH)  # L1 psum gate cols: i,g,f,o

# packed-weight column offsets (fp16, [128, WCOLS])
OFF_W1 = 0                      # 4 x [128,128] L1 input, order i,g2,f,o
OFF_WH1 = OFF_W1 + 4 * H1       # 4 x [128,128] L1 hidden
OFF_W2 = OFF_WH1 + 4 * H1       # 4 x [128,64]  L2 input (used for both Q halves)
OFF_WH2 = OFF_W2 + 4 * H2       # 4 x [128,128] L2 hidden, block-diag
OFF_WD = OFF_WH2 + 4 * 2 * H2   # [128,128] dense, block-diag
OFF_B1 = OFF_WD + 2 * OUT       # [1, 4*128] L1 bias
OFF_B2 = OFF_B1 + 4 * H1        # [1, 4*128] L2 bias, per gate [b|b]
OFF_BD = OFF_B2 + 4 * 2 * H2    # [1, 128] dense bias [bd|bd]
OFF_ONES = OFF_BD + 2 * OUT     # [1, 256] ones
WCOLS = OFF_ONES + BC           # 2624
WCRIT = OFF_W2                  # step-0-critical slice (L1 weights)

_PROGRAM_CACHE = {}


def build_program():
    if "nc" in _PROGRAM_CACHE:
        return _PROGRAM_CACHE["nc"]

    nc = bacc.Bacc(
        "TRN2", target_bir_lowering=False, debug=False,
        enable_asserts=False, num_devices=N_CORES,
    )

    xT_d = nc.declare_dram_parameter("xT", [D, T, BC], F16, isOutput=False)
    wpack_d = nc.declare_dram_parameter("wpack", [D, WCOLS], F16, isOutput=False)
    out_d = nc.declare_dram_parameter("outQ", [2 * H2, 2, QH], FP, isOutput=True)

    with TileContext(nc, num_cores=N_CORES) as tc:
        with (
            tc.tile_pool(name="const", bufs=1) as cpool,
            tc.tile_pool(name="sg", bufs=4) as sgpool,
            tc.tile_pool(name="acts", bufs=6) as apool,
            tc.tile_pool(name="state", bufs=8) as spool,
            tc.tile_pool(name="psA", bufs=2, space="PSUM") as psApool,
            tc.tile_pool(name="psB", bufs=2, space="PSUM") as psBpool,
            tc.tile_pool(name="ps2", bufs=2, space="PSUM") as ps2pool,
        ):
            # ---- constants / weights: batched DMAs
            wpack = cpool.tile([D, WCOLS], F16, tag="wpack")
            xs = cpool.tile([D, T, BC], F16, tag="xs")
            a0, b0 = X_CHUNKS[0]
            nc.sync.dma_start(out=xs[:, a0:b0, :], in_=xT_d[:, a0:b0, :])
            nc.sync.dma_start(out=wpack[:, 0:WCRIT], in_=wpack_d[:, 0:WCRIT])
            nc.sync.dma_start(out=wpack[:, OFF_B1:WCOLS], in_=wpack_d[:, OFF_B1:WCOLS])
            nc.sync.dma_start(out=wpack[:, WCRIT:OFF_B1], in_=wpack_d[:, WCRIT:OFF_B1])
            for a, b_ in X_CHUNKS[1:]:
                nc.sync.dma_start(out=xs[:, a:b_, :], in_=xT_d[:, a:b_, :])

            w1c = lambda j: wpack[:, OFF_W1 + j * H1:OFF_W1 + (j + 1) * H1]
            wh1c = lambda j: wpack[:, OFF_WH1 + j * H1:OFF_WH1 + (j + 1) * H1]
            w2c = lambda g: wpack[:, OFF_W2 + g * H2:OFF_W2 + (g + 1) * H2]
            wh2c = lambda g: wpack[:, OFF_WH2 + g * 2 * H2:OFF_WH2 + (g + 1) * 2 * H2]
            wdA = wpack[:, OFF_WD:OFF_WD + 2 * OUT]
            b1c = lambda j: wpack[0:1, OFF_B1 + j * H1:OFF_B1 + (j + 1) * H1]
            b2c = lambda g: wpack[0:1, OFF_B2 + g * 2 * H2:OFF_B2 + (g + 1) * 2 * H2]
            bdA = wpack[0:1, OFF_BD:OFF_BD + 2 * OUT]
            ones = lambda n: wpack[0:1, OFF_ONES:OFF_ONES + n]

            czero = cpool.tile([H1, CH], F16, tag="czero")
            nc.vector.memset(czero[:], 0.0)

            pools = {0: psApool, 1: psBpool}
            pg = {0: {}, 1: {}}    # L1 gate psum tile per chunk per step
            p2 = {}                # L2 gate psum tile per L2-step (both chunks)
            h1s = {0: {}, 1: {}}   # h1s[ch][t]
            cc1 = {0: {}, 1: {}}   # cc1[ch][t]: c'1 state
            cc2 = {}               # cc2[s] [128, 2, QH]: c'2 state, both chunks
            sg2s = {}              # sg2s[s]: L2 sigmoid outputs (o-gate kept)
            h2s = {0: {}, 1: {}}   # h2s[ch][u]

            def new_ptile(ch, t):
                p = pools[ch].tile([H1, 4 * CH], FP, tag=f"p{ch}")
                pg[ch][t] = p
                return p

            def l1_in_bias(ch, t, first=False):
                # start=True ONLY on the first mm into the tile: start clears
                # the whole psum bank's has_written flags, so issuing it per
                # column would wipe previously-written columns.
                p = pg[ch][t]
                xt = xs[:, t, ch * CH:(ch + 1) * CH]
                for j in range(4):
                    nc.tensor.matmul(p[:, L1G(j)], w1c(j), xt,
                                     start=(j == 0), stop=False)
                for j in range(4):
                    nc.tensor.matmul(p[:, L1G(j)], b1c(j), ones(CH),
                                     start=False, stop=(first and j == 3))

            for ch in (0, 1):
                new_ptile(ch, 0)
                l1_in_bias(ch, 0, first=True)

            for t in range(T + 3):
                s = t - 2   # L2 gate/cell step this iteration
                u = t - 3   # h2-update step this iteration

                # ================= PE =================
                # 1) L2 input+bias mms for s (deps are >=1 iteration old)
                if 0 <= s < T:
                    p2t = ps2pool.tile([H1, 2, 4, QH], FP, tag="p2",
                                       name="p2t")
                    p2[s] = p2t
                    # bias first: full-partition mms own the bank's single
                    # start; the 64-partition Q-half input mms then accumulate.
                    for ch in (0, 1):
                        for g in range(4):
                            nc.tensor.matmul(p2t[:, ch, g, :], b2c(g),
                                             ones(QH),
                                             start=(ch == 0 and g == 0),
                                             stop=False)
                    for ch in (0, 1):
                        h1v = h1s[ch][s]
                        for g in range(4):
                            nc.tensor.matmul(p2t[0:H2, ch, g, :], w2c(g),
                                             h1v[:, 0:QH],
                                             start=False, stop=False)
                            nc.tensor.matmul(p2t[H2:2 * H2, ch, g, :], w2c(g),
                                             h1v[:, QH:CH],
                                             start=False, stop=False)
                # 2) L1 hidden mms (chain-critical; h1n(t-1) just landed)
                if 0 < t < T:
                    for ch in (0, 1):
                        p, h1p = pg[ch][t], h1s[ch][t - 1]
                        for j in range(4):
                            nc.tensor.matmul(p[:, L1G(j)], wh1c(j), h1p,
                                             start=False, stop=(j == 3))
                # 3) L1 input+bias prefetch for t+1
                if t + 1 < T:
                    for ch in (0, 1):
                        new_ptile(ch, t + 1)
                        l1_in_bias(ch, t + 1)

                # ================= ACT: L1 gate sigmoids =====================
                sg1 = {}
                for ch in (0, 1):
                    if t < T:
                        sgt = sgpool.tile([H1, 4 * CH], F16, tag=f"sg{ch}",
                                          name=f"sg{ch}")
                        nc.scalar.activation(sgt[:], pg[ch][t][:], AF.Sigmoid)
                        sg1[ch] = sgt
                # ---- ACT: c'2 sigmoid for u (both chunks, one instr)
                thc2 = None
                if 0 <= u < T:
                    thc2 = apool.tile([H1, 2, QH], F16, tag="thc2",
                                      name="thc2")
                    nc.scalar.activation(thc2[:], cc2[u][:], AF.Sigmoid)

                # ================= DVE: L1 cells =============================
                for ch in (0, 1):
                    if t < T:
                        sgt = sg1[ch]
                        t1 = apool.tile([H1, CH], F16, tag=f"t1{ch}")
                        v = apool.tile([H1, CH], F16, tag=f"v{ch}")
                        c1n = spool.tile([H1, CH], F16, tag=f"c1{ch}",
                                         name=f"c1{ch}")
                        c1p = cc1[ch][t - 1] if t > 0 else czero
                        nc.vector.scalar_tensor_tensor(
                            t1[:], sgt[:, L1G(1)], 0.5, sgt[:, L1G(0)],
                            op0=ALU.subtract, op1=ALU.mult)
                        nc.vector.tensor_tensor(v[:], sgt[:, L1G(2)], c1p[:],
                                                op=ALU.mult)
                        nc.vector.scalar_tensor_tensor(
                            c1n[:], t1[:], 4.0, v[:],
                            op0=ALU.mult, op1=ALU.add)
                        cc1[ch][t] = c1n

                # ---- ACT: thc1 per chunk
                thc1 = {}
                for ch in (0, 1):
                    if t < T:
                        th = apool.tile([H1, CH], F16, tag=f"thc1{ch}",
                                        name=f"thc1{ch}")
                        nc.scalar.activation(th[:], cc1[ch][t][:], AF.Sigmoid)
                        thc1[ch] = th

                # ================= Pool: h2 updates for u ====================
                if 0 <= u < T:
                    sgo2 = sg2s[u]
                    for ch in (0, 1):
                        h2n = spool.tile([H1, QH], F16, tag=f"h2{ch}",
                                         name=f"h2{ch}")
                        nc.vector.scalar_tensor_tensor(
                            h2n[:], thc2[:, ch, :], 0.5, sgo2[:, ch, 3, :],
                            op0=ALU.subtract, op1=ALU.mult)
                        h2s[ch][u] = h2n

                # ================= PE: L2 hidden mms for s (queued last on PE;
                # their input h2n(s-1) lands mid-iteration on Pool) ===========
                if 0 <= s < T:
                    for ch in (0, 1):
                        h2v = h2s[ch][s - 1][:] if s > 0 else czero[:, 0:QH]
                        for g in range(4):
                            nc.tensor.matmul(p2[s][:, ch, g, :], wh2c(g),
                                             h2v, start=False,
                                             stop=(ch == 1 and g == 3))

                # ---- ACT: L2 sigmoid for s (both chunks, one instr; emitted
                # after the L2 hidden mms = its last psum writers)
                if 0 <= s < T:
                    sg2 = sgpool.tile([H1, 2, 4, QH], F16, tag="sg2",
                                      name="sg2")
                    nc.scalar.activation(sg2[:], p2[s][:], AF.Sigmoid)
                    sg2s[s] = sg2

                # ================= DVE: h1 updates =================
                for ch in (0, 1):
                    if t < T:
                        h1n = spool.tile([H1, CH], F16, tag=f"h1{ch}")
                        nc.vector.scalar_tensor_tensor(
                            h1n[:], thc1[ch][:], 0.5, sg1[ch][:, L1G(3)],
                            op0=ALU.subtract, op1=ALU.mult)
                        h1s[ch][t] = h1n

                # ================= Pool: L2 cells for s ======================
                if 0 <= s < T:
                    sg2 = sg2s[s]
                    c2n = spool.tile([H1, 2, QH], F16, tag="c2", name="c2")
                    for ch in (0, 1):
                        t2 = apool.tile([H1, QH], F16, tag=f"t2{ch}")
                        v2 = apool.tile([H1, QH], F16, tag=f"v2{ch}")
                        c2p = cc2[s - 1][:, ch, :] if s > 0 else czero[:, 0:QH]
                        nc.vector.scalar_tensor_tensor(
                            t2[:], sg2[:, ch, 1, :], 0.5, sg2[:, ch, 0, :],
                            op0=ALU.subtract, op1=ALU.mult)
                        nc.vector.tensor_tensor(v2[:], sg2[:, ch, 2, :], c2p,
                                                op=ALU.mult)
                        nc.vector.scalar_tensor_tensor(
                            c2n[:, ch, :], t2[:], 4.0, v2[:],
                            op0=ALU.mult, op1=ALU.add)
                    cc2[s] = c2n

                # ---- recycle stale refs
                for ch in (0, 1):
                    h1s[ch].pop(t - 3, None)
                    cc1[ch].pop(t - 2, None)
                    h2s[ch].pop(u - 2, None)
                cc2.pop(s - 2, None)
                sg2s.pop(u, None)
                p2.pop(s, None)
                pg[0].pop(t, None)
                pg[1].pop(t, None)

            # ---- dense head on h2[T-1] (per chunk); reuse L1 psum pools
            for ch in (0, 1):
                pd = new_ptile(ch, T + 10)
                nc.tensor.matmul(pd[0:2 * H2, 0:QH], bdA, ones(QH),
                                 start=True, stop=False)
                nc.tensor.matmul(pd[0:2 * H2, 0:QH], wdA, h2s[ch][T - 1],
                                 start=False, stop=True)
                outs = cpool.tile([2 * H2, QH], FP, tag=f"outs{ch}")
                nc.scalar.activation(outs[:], pd[0:2 * H2, 0:QH], AF.Relu)
                nc.sync.dma_start(out=out_d[:, ch, :], in_=outs[:])

    nc.finalize()
    _PROGRAM_CACHE["nc"] = nc
    return nc


def _prep_inputs(x, w_ih1, w_hh1, b_ih1, b_hh1, w_ih2, w_hh2, b_ih2, b_hh2,
                 w_dense, b_dense):
    """Host-side layout prep (fp16). Gate order on device: i, 2g, f, o.
    g weights/biases doubled because tanh(z) = 2*sigmoid(2z) - 1 on device.
    h is stored halved on device -> all weights consuming h are doubled."""
    f16 = np.float16

    def gates(w_t, H):  # w_t: [in, 4H] torch order (i,f,g,o) -> (i,2g,f,o)
        i, f, g, o = (np.float64(w_t[:, k * H:(k + 1) * H]) for k in range(4))
        return i, 2.0 * g, f, o

    wpack = np.zeros((D, WCOLS), f16)
    for off, w in ((OFF_W1, w_ih1.T), (OFF_WH1, 2.0 * np.float64(w_hh1.T))):
        for j, gw in enumerate(gates(w, H1)):
            wpack[:, off + j * H1:off + (j + 1) * H1] = gw.astype(f16)
    for j, gb in enumerate(gates((b_ih1 + b_hh1)[None, :], H1)):
        wpack[0:1, OFF_B1 + j * H1:OFF_B1 + (j + 1) * H1] = gb.astype(f16)

    # L2 input: plain [128, 64] per gate; hidden: block-diag [128, 128]
    for g, gw in enumerate(gates(2.0 * np.float64(w_ih2.T), H2)):
        wpack[:, OFF_W2 + g * H2:OFF_W2 + (g + 1) * H2] = gw.astype(f16)
    for g, gw in enumerate(gates(2.0 * np.float64(w_hh2.T), H2)):
        bd = np.zeros((D, 2 * H2))
        bd[0:H2, 0:H2] = gw
        bd[H2:2 * H2, H2:2 * H2] = gw
        wpack[:, OFF_WH2 + g * 2 * H2:OFF_WH2 + (g + 1) * 2 * H2] = bd.astype(f16)
    for g, gb in enumerate(gates((b_ih2 + b_hh2)[None, :], H2)):
        bb = np.concatenate([gb, gb], axis=-1)
        wpack[0:1, OFF_B2 + g * 2 * H2:OFF_B2 + (g + 1) * 2 * H2] = bb.astype(f16)

    # dense: block-diag [128, 128] (doubled: consumes h2), bias [bd|bd]
    wd = 2.0 * np.float64(w_dense.T)
    bdm = np.zeros((D, 2 * OUT))
    bdm[0:H2, 0:OUT] = wd
    bdm[H2:2 * H2, OUT:2 * OUT] = wd
    wpack[:, OFF_WD:OFF_WD + 2 * OUT] = bdm.astype(f16)
    wpack[0:1, OFF_BD:OFF_BD + OUT] = b_dense.astype(f16)[None, :]
    wpack[0:1, OFF_BD + OUT:OFF_BD + 2 * OUT] = b_dense.astype(f16)[None, :]
    wpack[0:1, OFF_ONES:OFF_ONES + BC] = 1.0

    xT = np.asarray(x, dtype=f16).transpose(2, 1, 0)  # [D,T,B]
    shared = dict(wpack=wpack)
    in_maps = []
    for c in range(N_CORES):
        m = dict(shared)
        m["xT"] = np.ascontiguousarray(xT[:, :, c * BC:(c + 1) * BC])
        in_maps.append(m)
    return in_maps


def _unshard(res):
    """outQ [128, 2, 64] per core -> [BC, OUT]; concat cores -> [B, OUT]."""
    outs = []
    for c in range(N_CORES):
        q = np.asarray(res.results[c]["outQ"])  # [2*H2, 2, QH]
        per = np.empty((BC, OUT), np.float32)
        for ch in range(2):
            for half in range(2):
                blk = q[half * OUT:(half + 1) * OUT, ch, :]  # [OUT, QH]
                b0 = ch * CH + half * QH
                per[b0:b0 + QH, :] = blk.T
        outs.append(per)
    return np.concatenate(outs, axis=0)


def _run(inputs, trace=False, **kw):
    nc = build_program()
    in_maps = _prep_inputs(**inputs)
    res = run_bass_kernel_spmd(nc, in_maps, list(range(N_CORES)), trace=trace, **kw)
    return _unshard(res).astype(np.float32), res


def kernel(**inputs):
    out, _ = _run(inputs, trace=False)
    return out


if __name__ == "__main__":
    import reference
    inputs = {k: np.asarray(v) for k, v in reference.setup_inputs().items()}
    expected = np.asarray(reference.reference(**inputs))
    out, res = _run(inputs, trace=os.environ.get("KTRACE", "0") == "1")
    err = np.abs(out - expected)
    rel = err.max() / (np.abs(expected).max() + 1e-12)
    print("max abs err:", err.max(), "rel:", rel)
    print("exec_time_ns:", res.exec_time_ns)
